# revision 3
# baseline (speedup 1.0000x reference)
"""BiLSTM-CRF sequence tagging loss on 8 Trainium2 NeuronCores.

Data-parallel: batch 128 sharded 16/core across 8 cores; each core runs the
full model (embedding gather, 2 BiLSTM layers, FC, CRF forward algorithm)
on its own shard with zero cross-core communication. Host sums the 8
per-core partial losses.

Key device-side tricks:
  - sigmoid-free LSTM cell: i,f,o weight rows pre-scaled by 0.5 so a single
    tanh activation covers all four gates (sigmoid(x) = 0.5*tanh(x/2)+0.5),
    with doubled cell/hidden state (C^=2c, h^=2h) so the whole pointwise
    update is exactly 4 fused scalar_tensor_tensor DVE ops per step.
  - xg (input projection) folded into PSUM via an identity matmul so the
    gate pre-activation needs no extra elementwise add.
  - CRF partition function in the exp domain (scaled forward algorithm):
    per step one 20x20 stationary matmul + one fused DVE multiply; 1/20
    folded into exp(emissions); periodic data-dependent rescale via a
    ones-matmul partition reduction, log-corrections accumulated on device.
"""

import numpy as np

V, E, H, C = 50000, 300, 256, 20
B, T_FULL = 128, 512
N_CORES = 8
B_LOC = B // N_CORES  # 16
G4 = 4 * H  # 1024
RESCALE_EVERY = 24

_COMPILED = {}


def _build(T, debug=False, phases='ABCDEF', reps=1):
    import concourse.bass as bass
    import concourse.mybir as mybir
    import concourse.tile as tile
    from concourse import bacc
    from concourse.masks import make_identity
    from contextlib import ExitStack

    f32 = mybir.dt.float32
    bf16 = mybir.dt.bfloat16
    i32 = mybir.dt.int32
    AF = mybir.ActivationFunctionType
    OP = mybir.AluOpType

    NTOK = T * B_LOC            # tokens per core
    NM = NTOK // 128            # 128-token m-tiles
    E_CH = [(0, 128), (128, 128), (256, 44)]   # E=300 chunks

    nc = bacc.Bacc("TRN2", debug=False, num_devices=N_CORES)

    def din(name, shape, dt=f32):
        return nc.dram_tensor(name, shape, dt, kind="ExternalInput").ap()

    ids_d = din("ids", (128, NM), i32)
    emb_d = din("emb", (V, E))
    w0i_d = din("w0i", (2, E, G4))
    w0h_d = din("w0h", (2, H, G4))
    b0_d = din("b0r", (128, 2, G4))
    w1i_d = din("w1i", (2, 2 * H, G4))
    w1h_d = din("w1h", (2, H, G4))
    b1_d = din("b1r", (128, 2, G4))
    fct_d = din("fcT", (2 * H, C))
    fcb_d = din("fcbr", (128, C))
    mask_d = din("maskE", (128, NM * C))
    sel_d = din("selm", (128, B_LOC))
    pm_d = din("Pm", (C, C))
    est_d = din("estart", (C, 1))
    een_d = din("eend", (C, 1))
    chain_d = din("chain", (1, 1))

    s_out = nc.dram_tensor("S_out", (1, B_LOC), f32, kind="ExternalOutput").ap()
    ne_out = nc.dram_tensor("numE_out", (1, B_LOC), f32, kind="ExternalOutput").ap()
    la_out = nc.dram_tensor("logacc_out", (1, B_LOC), f32, kind="ExternalOutput").ap()

    # DRAM scratch
    xg0_d = nc.dram_tensor("xg0", (2, NTOK, G4), f32).ap()
    xg1_d = nc.dram_tensor("xg1", (2, NTOK, G4), f32).ap()
    dbg_kind = "ExternalOutput" if debug else "Internal"
    hc0_d = nc.dram_tensor("hcat0", (T, B_LOC, 2 * H), f32, kind=dbg_kind).ap()
    hc1_d = nc.dram_tensor("hcat1", (T, B_LOC, 2 * H), f32, kind=dbg_kind).ap()

    with tile.TileContext(nc) as tc, ExitStack() as top:
        cp = top.enter_context(tc.tile_pool(name="const", bufs=1))

        ident = cp.tile([128, 128], f32)
        make_identity(nc, ident[:])
        id16f = cp.tile([16, 16], f32)
        make_identity(nc, id16f[:])
        id16b = cp.tile([16, 16], bf16)
        nc.vector.tensor_copy(id16b[:], id16f[:])
        # identity replicated at partition offsets 0 and 32 (for dir-1 transposes)
        id64f = cp.tile([64, 16], f32)
        nc.gpsimd.memset(id64f[:], 0.0)
        make_identity(nc, id64f[0:16, :], nomemset=True)
        make_identity(nc, id64f[32:48, :], nomemset=True)

        ids_sb = cp.tile([128, NM], i32)
        nc.sync.dma_start(ids_sb[:], ids_d[:])

        # weight staging is per-phase (SBUF freed between phases)
        def load_w(pool, dram, rows, name):
            out = []
            for d in range(2):
                chs = []
                r0 = 0
                while r0 < rows:
                    ck = min(128, rows - r0)
                    t = pool.tile([ck, G4], f32, tag=f"{name}{d}_{r0}", name=f"{name}{d}_{r0}")
                    nc.sync.dma_start(t[:], dram[d, r0 : r0 + ck, :])
                    chs.append((t, ck))
                    r0 += ck
                out.append(chs)
            return out
        fct_sb = []
        for k in range(4):
            t = cp.tile([128, C], f32, tag=f"fct{k}")
            nc.sync.dma_start(t[:], fct_d[k * 128 : (k + 1) * 128, :])
            fct_sb.append(t)
        fcb_sb = cp.tile([128, C], f32)
        nc.sync.dma_start(fcb_sb[:], fcb_d[:])
        mask_sb = cp.tile([128, NM * C], f32)
        nc.sync.dma_start(mask_sb[:], mask_d[:])
        sel_sb = cp.tile([128, B_LOC], f32)
        nc.sync.dma_start(sel_sb[:], sel_d[:])
        pm_sb = cp.tile([C, C], f32)
        nc.sync.dma_start(pm_sb[:], pm_d[:])
        est_sb = cp.tile([C, 1], f32)
        nc.sync.dma_start(est_sb[:], est_d[:])
        een_sb = cp.tile([C, 1], f32)
        nc.sync.dma_start(een_sb[:], een_d[:])
        ones20 = cp.tile([C, C], f32)
        nc.vector.memset(ones20[:], 1.0)
        chain_sb = cp.tile([1, 1], f32)
        nc.sync.dma_start(chain_sb[:], chain_d[:])

        def whole_model():
            # ---------------- Phase A: gather + transpose + L0 input proj ----------
            if 'A' in phases:
              with nc.named_scope("phaseA"), ExitStack() as es:
                sp = es.enter_context(tc.tile_pool(name="pA", bufs=3))
                wp = es.enter_context(tc.tile_pool(name="pAw", bufs=1))
                pp = es.enter_context(tc.tile_pool(name="pAp", bufs=2, space="PSUM"))
                w0i_sb = load_w(wp, w0i_d, E, "w0i")
                b0_sb = wp.tile([128, 2 * G4], f32)
                nc.sync.dma_start(b0_sb[:], b0_d[:].rearrange("p a b -> p (a b)"))
                for m in range(NM):
                    xm = sp.tile([128, E], f32, tag="xm")
                    nc.gpsimd.indirect_dma_start(
                        out=xm[:],
                        out_offset=None,
                        in_=emb_d[:],
                        in_offset=bass.IndirectOffsetOnAxis(ap=ids_sb[:, m : m + 1], axis=0),
                    )
                    xTm = []
                    for (r0, ck) in E_CH:
                        tp = pp.tile([128, 128], f32, tag="tp")
                        nc.tensor.transpose(out=tp[:ck, :], in_=xm[:, r0 : r0 + ck], identity=ident[:])
                        xt = sp.tile([128, 128], f32, tag=f"xt{r0}")
                        nc.scalar.copy(out=xt[:ck, :], in_=tp[:ck, :])
                        xTm.append((xt, ck))
                    for d in range(2):
                        ps = pp.tile([128, G4], f32, tag="psxg")
                        for ki, (xt, ck) in enumerate(xTm):
                            for nb in range(2):
                                nc.tensor.matmul(
                                    ps[:, nb * 512 : (nb + 1) * 512],
                                    lhsT=xt[:ck, :],
                                    rhs=w0i_sb[d][ki][0][:ck, nb * 512 : (nb + 1) * 512],
                                    start=(ki == 0),
                                    stop=(ki == 2),
                                )
                        ev = sp.tile([128, G4], f32, tag="ev")
                        nc.vector.scalar_tensor_tensor(
                            out=ev[:], in0=ps[:], scalar=0.0,
                            in1=b0_sb[:, d * G4 : (d + 1) * G4],
                            op0=OP.add, op1=OP.add,
                        )
                        nc.sync.dma_start(xg0_d[d, m * 128 : (m + 1) * 128, :], ev[:])

            # ---------------- LSTM recurrence (shared for L0/L1) -------------------
            # Two fully independent per-direction chains so their ~7-hop
            # cross-engine dependency latencies overlap.
            def recurrence(xg_d, wh_d, hout_d):
                with ExitStack() as es:
                    rp = es.enter_context(tc.tile_pool(name="rec", bufs=1))
                    sp = es.enter_context(tc.tile_pool(name="recw", bufs=4))
                    gp = es.enter_context(tc.tile_pool(name="recG", bufs=1, space="PSUM"))
                    tp = es.enter_context(tc.tile_pool(name="recT", bufs=4, space="PSUM"))
                    wh_sb = load_w(rp, wh_d, H, "wh")

                    G = [gp.tile([16, G4], f32, tag=f"G{d}", name=f"G{d}") for d in range(2)]
                    Cst = [rp.tile([16, H], f32, tag=f"C{d}", name=f"C{d}") for d in range(2)]
                    hT = [[rp.tile([128, 16], f32, tag=f"hT{d}_{k}", name=f"hT{d}_{k}") for k in range(2)] for d in range(2)]
                    for d in range(2):
                        nc.vector.memset(Cst[d][:], 0.0)
                        for k in range(2):
                            nc.vector.memset(hT[d][k][:], 0.0)

                    def step(d, t):
                        tt = t if d == 0 else T - 1 - t
                        g = G[d]
                        xgt = sp.tile([16, G4], f32, tag=f"xgt{d}", name=f"xgt{d}")
                        nc.sync.dma_start(xgt[:], xg_d[d, tt * 16 : (tt + 1) * 16, :])
                        for nb in range(2):
                            nc.tensor.matmul(
                                g[:, nb * 512 : (nb + 1) * 512],
                                lhsT=id16f[:],
                                rhs=xgt[:, nb * 512 : (nb + 1) * 512],
                                start=True, stop=False,
                            )
                        for k in range(2):
                            for nb in range(2):
                                nc.tensor.matmul(
                                    g[:, nb * 512 : (nb + 1) * 512],
                                    lhsT=hT[d][k][:],
                                    rhs=wh_sb[d][k][0][:, nb * 512 : (nb + 1) * 512],
                                    start=False, stop=(k == 1),
                                )
                        Tall = sp.tile([16, G4], f32, tag=f"Tall{d}", name=f"Tall{d}")
                        nc.scalar.activation(Tall[:], g[:], AF.Tanh)
                        A = sp.tile([16, H], f32, tag=f"A{d}", name=f"A{d}")
                        nc.vector.scalar_tensor_tensor(
                            out=A[:], in0=Tall[:, 256:512], scalar=1.0, in1=Cst[d][:],
                            op0=OP.add, op1=OP.mult)
                        Bv = sp.tile([16, H], f32, tag=f"Bv{d}", name=f"Bv{d}")
                        nc.vector.scalar_tensor_tensor(
                            out=Bv[:], in0=Tall[:, 0:256], scalar=1.0, in1=Tall[:, 512:768],
                            op0=OP.add, op1=OP.mult)
                        nc.vector.scalar_tensor_tensor(
                            out=Cst[d][:], in0=A[:], scalar=0.5, in1=Bv[:],
                            op0=OP.mult, op1=OP.add)
                        TC = sp.tile([16, H], f32, tag=f"TC{d}", name=f"TC{d}")
                        nc.scalar.activation(TC[:], Cst[d][:], AF.Tanh, scale=0.5)
                        Hh = sp.tile([16, H], f32, tag=f"Hh{d}", name=f"Hh{d}")
                        nc.vector.scalar_tensor_tensor(
                            out=Hh[:], in0=Tall[:, 768:1024], scalar=1.0, in1=TC[:],
                            op0=OP.add, op1=OP.mult)
                        for k in range(2):
                            tps = tp.tile([128, 16], f32, tag="tps", name="tps")
                            nc.tensor.transpose(
                                out=tps[:], in_=Hh[:, k * 128 : (k + 1) * 128],
                                identity=id16f[:])
                            nc.scalar.copy(out=hT[d][k][:], in_=tps[:])
                        nc.sync.dma_start(hout_d[tt, :, d * H : (d + 1) * H], Hh[:])

                    for t in range(T):
                        step(0, t)
                        step(1, t)

            if 'B' in phases:
                with nc.named_scope("phaseB"):
                    recurrence(xg0_d, w0h_d, hc0_d)

            # ---------------- Phase C: L1 input proj from hcat0 --------------------
            def proj_l1(hin_d, xgo_d):
                with ExitStack() as es:
                    sp = es.enter_context(tc.tile_pool(name="pC", bufs=3))
                    wp = es.enter_context(tc.tile_pool(name="pCw", bufs=1))
                    pp = es.enter_context(tc.tile_pool(name="pCp", bufs=2, space="PSUM"))
                    w1i_sb = load_w(wp, w1i_d, 2 * H, "w1i")
                    b1_sb = wp.tile([128, 2 * G4], f32)
                    nc.sync.dma_start(b1_sb[:], b1_d[:].rearrange("p a b -> p (a b)"))
                    for m in range(NM):
                        hm = sp.tile([128, 2 * H], f32, tag="hm")
                        nc.sync.dma_start(
                            hm[:], hin_d[:].rearrange("t b h -> (t b) h")[m * 128 : (m + 1) * 128, :])
                        hTm = []
                        for k in range(4):
                            tps = pp.tile([128, 128], f32, tag="tp")
                            nc.tensor.transpose(out=tps[:], in_=hm[:, k * 128 : (k + 1) * 128], identity=ident[:])
                            ht = sp.tile([128, 128], f32, tag=f"ht{k}")
                            nc.scalar.copy(out=ht[:], in_=tps[:])
                            hTm.append(ht)
                        for d in range(2):
                            ps = pp.tile([128, G4], f32, tag="psxg")
                            for k in range(4):
                                for nb in range(2):
                                    nc.tensor.matmul(
                                        ps[:, nb * 512 : (nb + 1) * 512],
                                        lhsT=hTm[k][:],
                                        rhs=w1i_sb[d][k][0][:, nb * 512 : (nb + 1) * 512],
                                        start=(k == 0), stop=(k == 3),
                                    )
                            ev = sp.tile([128, G4], f32, tag="ev")
                            nc.vector.scalar_tensor_tensor(
                                out=ev[:], in0=ps[:], scalar=0.0,
                                in1=b1_sb[:, d * G4 : (d + 1) * G4],
                                op0=OP.add, op1=OP.add)
                            nc.sync.dma_start(xgo_d[d, m * 128 : (m + 1) * 128, :], ev[:])

            if 'C' in phases:
                with nc.named_scope("phaseC"):
                    proj_l1(hc0_d, xg1_d)
            if 'D' in phases:
                with nc.named_scope("phaseD"):
                    recurrence(xg1_d, w1h_d, hc1_d)

            # ---------------- Phase E: FC -> emissions, numE, exp-emissions --------
            ET = cp.tile([C, NTOK], f32)          # exp(e)/C, transposed [C, tok]
            accT = cp.tile([128, NM], f32)
            nlnC = cp.tile([128, 1], f32)
            nc.vector.memset(nlnC[:], -float(np.log(C)))
            if 'E' in phases:
              with nc.named_scope("phaseE"), ExitStack() as es:
                sp = es.enter_context(tc.tile_pool(name="pE", bufs=3))
                pp = es.enter_context(tc.tile_pool(name="pEp", bufs=2, space="PSUM"))
                for m in range(NM):
                    hm = sp.tile([128, 2 * H], f32, tag="hm")
                    nc.sync.dma_start(
                        hm[:], hc1_d[:].rearrange("t b h -> (t b) h")[m * 128 : (m + 1) * 128, :])
                    ps = pp.tile([128, C], f32, tag="pse")
                    for k in range(4):
                        tps = pp.tile([128, 128], f32, tag="tp")
                        nc.tensor.transpose(out=tps[:], in_=hm[:, k * 128 : (k + 1) * 128], identity=ident[:])
                        ht = sp.tile([128, 128], f32, tag="ht")
                        nc.scalar.copy(out=ht[:], in_=tps[:])
                        nc.tensor.matmul(
                            ps[:], lhsT=ht[:], rhs=fct_sb[k][:],
                            start=(k == 0), stop=(k == 3))
                    em = sp.tile([128, C], f32, tag="em")
                    nc.vector.scalar_tensor_tensor(
                        out=em[:], in0=ps[:], scalar=0.0, in1=fcb_sb[:],
                        op0=OP.add, op1=OP.add)
                    junk = sp.tile([128, C], f32, tag="junk")
                    nc.vector.scalar_tensor_tensor(
                        out=junk[:], in0=em[:], scalar=0.0, in1=mask_sb[:, m * C : (m + 1) * C],
                        op0=OP.add, op1=OP.mult, accum_out=accT[:, m : m + 1])
                    ee = sp.tile([128, C], f32, tag="ee")
                    nc.scalar.activation(ee[:], em[:], AF.Exp, bias=nlnC[:, :1])
                    tps = pp.tile([128, 128], f32, tag="tp2")
                    nc.tensor.transpose(out=tps[:C, :], in_=ee[:], identity=ident[:])
                    nc.scalar.copy(out=ET[:, m * 128 : (m + 1) * 128], in_=tps[:C, :])

                # numE[b] = sum_p sel[p,b] * rowsum(accT)
                accR = sp.tile([128, 1], f32, tag="accR")
                nc.vector.tensor_reduce(accR[:], accT[:], axis=mybir.AxisListType.X, op=OP.add)
                psn = pp.tile([16, 1], f32, tag="psn")
                nc.tensor.matmul(psn[:], lhsT=sel_sb[:], rhs=accR[:], start=True, stop=True)
                neo = sp.tile([16, 1], f32, tag="neo")
                nc.scalar.copy(out=neo[:], in_=psn[:])
                nc.sync.dma_start(ne_out[:].rearrange("a b -> b a"), neo[:])

            # ---------------- Phase F: CRF forward algorithm (exp domain) ----------
            if 'F' not in phases:
                for out_ap in (s_out, ne_out, la_out):
                    pass
            if 'F' in phases:
              with nc.named_scope("phaseF"), ExitStack() as es:
                sp = es.enter_context(tc.tile_pool(name="pF", bufs=4))
                pp = es.enter_context(tc.tile_pool(name="pFp", bufs=2, space="PSUM"))
                logacc = cp.tile([1, B_LOC], f32)
                nc.vector.memset(logacc[:], 0.0)
                a = sp.tile([C, B_LOC], f32, tag="a0")
                nc.vector.tensor_scalar(a[:], ET[:, 0:B_LOC], est_sb[:, :1], None, op0=OP.mult)
                for t in range(1, T):
                    ps = pp.tile([C, B_LOC], f32, tag="psa")
                    nc.tensor.matmul(ps[:], lhsT=pm_sb[:], rhs=a[:], start=True, stop=True)
                    a = sp.tile([C, B_LOC], f32, tag=f"a{t % 3 + 1}")
                    nc.vector.scalar_tensor_tensor(
                        out=a[:], in0=ps[:], scalar=0.0,
                        in1=ET[:, t * B_LOC : (t + 1) * B_LOC],
                        op0=OP.add, op1=OP.mult)
                    if t % RESCALE_EVERY == 0:
                        nrm = pp.tile([C, B_LOC], f32, tag="nrm")
                        nc.tensor.matmul(nrm[:], lhsT=ones20[:], rhs=a[:], start=True, stop=True)
                        lnn = sp.tile([1, B_LOC], f32, tag="lnn")
                        nc.scalar.activation(lnn[:], nrm[:1, :], AF.Ln)
                        nc.vector.tensor_tensor(
                            out=logacc[:], in0=logacc[:], in1=lnn[:], op=OP.add)
                        rcp = sp.tile([C, B_LOC], f32, tag="rcp")
                        nc.vector.reciprocal(rcp[:], nrm[:])
                        a2 = sp.tile([C, B_LOC], f32, tag="a_rs")
                        nc.vector.tensor_tensor(out=a2[:], in0=a[:], in1=rcp[:], op=OP.mult)
                        a = a2
                af = sp.tile([C, B_LOC], f32, tag="af")
                nc.vector.tensor_scalar(af[:], a[:], een_sb[:, :1], None, op0=OP.mult)
                pss = pp.tile([1, B_LOC], f32, tag="pss")
                nc.tensor.matmul(pss[:], lhsT=ones20[:, :1], rhs=af[:], start=True, stop=True)
                so = sp.tile([1, B_LOC], f32, tag="so")
                nc.scalar.copy(out=so[:], in_=pss[:])
                nc.sync.dma_start(s_out[:], so[:])
                lao = sp.tile([1, B_LOC], f32, tag="lao")
                nc.vector.tensor_scalar(lao[:], logacc[:], chain_sb[:, :1], None, op0=OP.add)
                nc.sync.dma_start(la_out[:], lao[:])

        for _rep in range(reps):
            whole_model()

    nc.compile()
    return nc


def _prep_host(inputs, T):
    """Host-side weight transforms + per-core in_maps."""
    f32 = np.float32
    ids_full = np.asarray(inputs["input_ids"]).astype(np.int32)      # [B, T]
    labels = np.asarray(inputs["labels"]).astype(np.int64)           # [B, T]
    emb = np.asarray(inputs["emb"], dtype=f32)
    trans = np.asarray(inputs["transitions"], dtype=f32)
    start = np.asarray(inputs["start_trans"], dtype=f32)
    end = np.asarray(inputs["end_trans"], dtype=f32)

    colscale = np.ones(G4, f32)
    colscale[0:256] = 0.5       # i
    colscale[256:512] = 0.5     # f
    colscale[768:1024] = 0.5    # o

    def prep_layer(wi, wh, bi, bh, in_scale):
        # wi [2, 4H, in], wh [2, 4H, H] -> transposed, scaled
        wiT = np.ascontiguousarray(np.transpose(wi, (0, 2, 1))).astype(f32)
        whT = np.ascontiguousarray(np.transpose(wh, (0, 2, 1))).astype(f32)
        wiT = wiT * in_scale * colscale[None, None, :]
        whT = whT * 0.5 * colscale[None, None, :]
        b = (np.asarray(bi, f32) + np.asarray(bh, f32)) * colscale[None, :]
        return wiT, whT, b

    w0i, w0h, b0 = prep_layer(inputs["w_ih_l0"], inputs["w_hh_l0"],
                              inputs["b_ih_l0"], inputs["b_hh_l0"], 1.0)
    w1i, w1h, b1 = prep_layer(inputs["w_ih_l1"], inputs["w_hh_l1"],
                              inputs["b_ih_l1"], inputs["b_hh_l1"], 0.5)
    fcT = (np.asarray(inputs["fc_w"], f32).T * 0.5).astype(f32)      # [2H, C]
    fcb = np.asarray(inputs["fc_b"], f32)

    b0r = np.broadcast_to(b0[None, :, :], (128, 2, G4)).copy()
    b1r = np.broadcast_to(b1[None, :, :], (128, 2, G4)).copy()
    fcbr = np.broadcast_to(fcb[None, :], (128, C)).copy()
    Pm = np.exp(trans).astype(f32)
    est = np.exp(start).astype(f32).reshape(C, 1)
    een = np.exp(end).astype(f32).reshape(C, 1)
    selm = (np.arange(128)[:, None] % B_LOC == np.arange(B_LOC)[None, :]).astype(f32)

    NTOK = T * B_LOC
    NM = NTOK // 128
    in_maps = []
    host_num = np.zeros(B, np.float64)
    for c in range(N_CORES):
        bs = slice(c * B_LOC, (c + 1) * B_LOC)
        ids_c = ids_full[bs, :T].T.reshape(NTOK)                     # (t,b) t-major
        ids_tile = ids_c.reshape(NM, 128).T.copy()                   # [128, NM]
        lab_c = labels[bs, :T].T.reshape(NTOK)                       # token (t,b)
        maskE = np.zeros((128, NM * C), f32)
        toks = np.arange(NTOK)
        maskE[toks % 128, (toks // 128) * C + lab_c] = 1.0
        in_maps.append({
            "ids": ids_tile.astype(np.int32), "emb": emb,
            "w0i": w0i, "w0h": w0h, "b0r": b0r,
            "w1i": w1i, "w1h": w1h, "b1r": b1r,
            "fcT": fcT, "fcbr": fcbr, "maskE": maskE, "selm": selm,
            "Pm": Pm, "estart": est, "eend": een,
            "chain": np.zeros((1, 1), f32),
        })
        # host part of gold-path score (pure input data)
        lb = labels[bs, :T]
        host_num[c * B_LOC:(c + 1) * B_LOC] = (
            start[lb[:, 0]].astype(np.float64)
            + trans[lb[:, :-1], lb[:, 1:]].sum(-1)
            + end[lb[:, -1]]
        )
    return in_maps, host_num


def _run(inputs, T):
    from concourse.bass_utils import run_bass_kernel_spmd

    if T not in _COMPILED:
        _COMPILED[T] = _build(T)
    nc = _COMPILED[T]
    in_maps, host_num = _prep_host(inputs, T)
    res = run_bass_kernel_spmd(nc, in_maps, core_ids=list(range(N_CORES)))
    total = 0.0
    for c in range(N_CORES):
        r = res.results[c]
        S = r["S_out"].reshape(B_LOC).astype(np.float64)
        numE = r["numE_out"].reshape(B_LOC).astype(np.float64)
        logacc = r["logacc_out"].reshape(B_LOC).astype(np.float64)
        logZ = np.log(S) + logacc + T * np.log(C)
        num = host_num[c * B_LOC:(c + 1) * B_LOC] + numE
        total += (logZ - num).sum()
    return np.float32(total)


def kernel(**inputs):
    return _run(inputs, T_FULL)



# revision 13
# speedup vs baseline: 63.5141x; 63.5141x over previous
"""BiLSTM-CRF sequence tagging loss on 8 Trainium2 NeuronCores.

Data-parallel: batch 128 sharded 16/core across 8 cores; each core runs the
full model (embedding gather, 2 BiLSTM layers, FC, CRF forward algorithm)
on its own shard with zero cross-core communication. Host sums the 8
per-core partial losses.

v2 design notes (vs the fp32 baseline):
  - All matmul streams in bf16 (weights, xg scratch, h scratch); fp32 only
    for the cell state C and the CRF.
  - The two LSTM directions run as two fully independent dependency chains
    (per-dir instructions) so their ~8-hop cross-engine serial latency
    overlaps; their matmuls run concurrently on the PE via column-group
    tiling (dir0 -> array cols 0:16, dir1 -> cols 32:48).
  - sigmoid-free LSTM cell (i,f,o pre-scaled 0.5; one tanh covers all four
    gates; state doubled C^=2c, h^=2h).
  - xg (input projection) folded into PSUM via an identity matmul; biases
    ride as an extra contraction row in the L0 projection.
  - h is stored to DRAM *transposed* (hcT[dir][chunk][128, (t,b)]) so the
    L1 input projection and the FC consume it directly as lhsT with no
    per-tile transposes.
  - xg DMA loads batched 8 steps per transfer; h stores batched 8 steps.
  - CRF partition function in the exp domain (scaled forward algorithm),
    split into two 8-sample chains to halve the serial-latency exposure.
"""

import numpy as np

V, E, H, C = 50000, 300, 256, 20
B, T_FULL = 128, 512
N_CORES = 8
B_LOC = B // N_CORES  # 16
G4 = 4 * H  # 1024
RESCALE_EVERY = 24
SPD = 4  # steps per DMA batch in the recurrence

_COMPILED = {}


def _build(T, debug=False, phases='ABCDEF', reps=1):
    import concourse.bass as bass
    import concourse.mybir as mybir
    import concourse.tile as tile
    from concourse import bacc
    from concourse.masks import make_identity
    from contextlib import ExitStack

    f32 = mybir.dt.float32
    bf16 = mybir.dt.bfloat16
    i32 = mybir.dt.int32
    AF = mybir.ActivationFunctionType
    OP = mybir.AluOpType

    NTOK = T * B_LOC            # tokens per core
    NM = NTOK // 128            # 128-token m-tiles
    assert T % SPD == 0

    nc = bacc.Bacc("TRN2", debug=False, num_devices=N_CORES)

    def din(name, shape, dt=f32):
        return nc.dram_tensor(name, shape, dt, kind="ExternalInput").ap()

    ids_d = din("ids", (128, NM), i32)
    emb_d = din("emb", (V, E))
    w0i_d = din("w0i", (2, E + 1, G4), bf16)   # row E = bias
    w0h_d = din("w0h", (2, H, G4), bf16)
    w1i_d = din("w1i", (2, 2 * H, G4), bf16)
    w1h_d = din("w1h", (2, H, G4), bf16)
    b1_d = din("b1r", (128, 2, G4))            # L1 bias, broadcast 128
    fct_d = din("fcT", (2 * H, C), bf16)
    fcb_d = din("fcbr", (128, C))
    mask_d = din("maskE", (128, NM * C))
    sel_d = din("selm", (128, B_LOC))
    pm_d = din("Pm", (C, C))
    est_d = din("estart", (C, 1))
    een_d = din("eend", (C, 1))
    chain_d = din("chain", (1, 1))

    s_out = nc.dram_tensor("S_out", (1, B_LOC), f32, kind="ExternalOutput").ap()
    ne_out = nc.dram_tensor("numE_out", (1, B_LOC), f32, kind="ExternalOutput").ap()
    la_out = nc.dram_tensor("logacc_out", (1, B_LOC), f32, kind="ExternalOutput").ap()

    # DRAM scratch (bf16)
    xg0_d = nc.dram_tensor("xg0", (2, NTOK, G4), bf16).ap()
    xg1_d = nc.dram_tensor("xg1", (2, NTOK, G4), bf16).ap()
    # transposed h storage: [dir, kchunk, 128 hrows, (t,b)]
    hcT0_d = nc.dram_tensor("hcT0", (2, 2, 128, NTOK), bf16).ap()
    hcT1_d = nc.dram_tensor("hcT1", (2, 2, 128, NTOK), bf16).ap()

    with tile.TileContext(nc) as tc, ExitStack() as top:
        cp = top.enter_context(tc.tile_pool(name="const", bufs=1))

        ident = cp.tile([128, 128], f32)
        make_identity(nc, ident[:])
        id16f = cp.tile([16, 16], f32)
        make_identity(nc, id16f[:])
        id16b = cp.tile([16, 16], bf16)
        nc.vector.tensor_copy(id16b[:], id16f[:])

        ids_sb = cp.tile([128, NM], i32)
        nc.sync.dma_start(ids_sb[:], ids_d[:])

        fct_sb = []
        for k in range(4):
            t = cp.tile([128, C], bf16, tag=f"fct{k}")
            nc.sync.dma_start(t[:], fct_d[k * 128 : (k + 1) * 128, :])
            fct_sb.append(t)
        fcb_sb = cp.tile([128, C], f32)
        nc.sync.dma_start(fcb_sb[:], fcb_d[:])
        mask_sb = cp.tile([128, NM * C], f32)
        nc.sync.dma_start(mask_sb[:], mask_d[:])
        sel_sb = cp.tile([128, B_LOC], f32)
        nc.sync.dma_start(sel_sb[:], sel_d[:])
        pm_sb = cp.tile([C, C], f32)
        nc.sync.dma_start(pm_sb[:], pm_d[:])
        est_sb = cp.tile([C, 1], f32)
        nc.sync.dma_start(est_sb[:], est_d[:])
        een_sb = cp.tile([C, 1], f32)
        nc.sync.dma_start(een_sb[:], een_d[:])
        ones20 = cp.tile([C, C], f32)
        nc.vector.memset(ones20[:], 1.0)
        chain_sb = cp.tile([1, 1], f32)
        nc.sync.dma_start(chain_sb[:], chain_d[:])

        def load_w(pool, dram, rows, name, dt=bf16):
            """Stage [2, rows, G4] DRAM weights as per-dir row-chunk tiles."""
            out = []
            for d in range(2):
                chs = []
                r0 = 0
                while r0 < rows:
                    ck = min(128, rows - r0)
                    t = pool.tile([ck, G4], dt, tag=f"{name}{d}_{r0}", name=f"{name}{d}_{r0}")
                    nc.sync.dma_start(t[:], dram[d, r0 : r0 + ck, :])
                    chs.append((t, ck))
                    r0 += ck
                out.append(chs)
            return out

        def whole_model():
            # ---------------- Phase A: gather + transpose + L0 input proj ----------
            # m-tiles in ping-pong order so both recurrence chain heads
            # (t=0 fwd, t=T-1 bwd) materialize early.
            if 'A' in phases:
              with nc.named_scope("phaseA"), ExitStack() as es:
                sp = es.enter_context(tc.tile_pool(name="pA", bufs=3))
                wp = es.enter_context(tc.tile_pool(name="pAw", bufs=1))
                pp = es.enter_context(tc.tile_pool(name="pAp", bufs=2, space="PSUM"))
                # chunk split keeps the bias ones-row at a 32-aligned partition
                E_CH = [(0, 128), (128, 108), (236, 65)]  # last holds bias row at 64
                w0i_sb = []
                for d in range(2):
                    chs = []
                    for (r0, ck) in E_CH:
                        t = wp.tile([ck, G4], bf16, tag=f"w0i{d}_{r0}", name=f"w0i{d}_{r0}")
                        nc.sync.dma_start(t[:], w0i_d[d, r0 : r0 + ck, :])
                        chs.append((t, ck))
                    w0i_sb.append(chs)
                order = []
                for i in range(NM):
                    order.append(i // 2 if i % 2 == 0 else NM - 1 - i // 2)
                for m in order:
                    xm = sp.tile([128, E], f32, tag="xm")
                    nc.gpsimd.indirect_dma_start(
                        out=xm[:],
                        out_offset=None,
                        in_=emb_d[:],
                        in_offset=bass.IndirectOffsetOnAxis(ap=ids_sb[:, m : m + 1], axis=0),
                    )
                    xTm = []
                    for (r0, ck) in E_CH:
                        dk = min(ck, E - r0)      # data rows in this chunk (64 for last)
                        tp = pp.tile([128, 128], f32, tag="tp")
                        nc.tensor.transpose(out=tp[:dk, :], in_=xm[:, r0 : r0 + dk], identity=ident[:])
                        xt = sp.tile([ck, 128], bf16, tag=f"xt{r0}")
                        nc.scalar.copy(out=xt[:dk, :], in_=tp[:dk, :])
                        if ck > dk:               # ones row for the bias
                            nc.vector.memset(xt[dk:ck, :], 1.0)
                        xTm.append((xt, ck))
                    for d in range(2):
                        ps = pp.tile([128, G4], f32, tag="psxg")
                        for ki, (xt, ck) in enumerate(xTm):
                            for nb in range(2):
                                nc.tensor.matmul(
                                    ps[:, nb * 512 : (nb + 1) * 512],
                                    lhsT=xt[:, :],
                                    rhs=w0i_sb[d][ki][0][:, nb * 512 : (nb + 1) * 512],
                                    start=(ki == 0),
                                    stop=(ki == 2),
                                )
                        ev = sp.tile([128, G4], bf16, tag=f"ev{d}")
                        if d == 0:
                            nc.scalar.copy(out=ev[:], in_=ps[:])
                        else:
                            nc.vector.tensor_copy(ev[:], ps[:])
                        nc.sync.dma_start(xg0_d[d, m * 128 : (m + 1) * 128, :], ev[:])

            # ---------------- LSTM recurrence (shared for L0/L1) -------------------
            # Two independent per-direction chains; dir0 matmuls on PE column
            # group 0 (psum partitions 0:16), dir1 on group 1 (partitions 32:48).
            def recurrence(xg_d, wh_d, houtT_d):
                with ExitStack() as es:
                    rp = es.enter_context(tc.tile_pool(name="rec", bufs=1))
                    xp = es.enter_context(tc.tile_pool(name="recx", bufs=2))
                    sp = es.enter_context(tc.tile_pool(name="recw", bufs=3))
                    gp = es.enter_context(tc.tile_pool(name="recG", bufs=1, space="PSUM"))
                    tp = es.enter_context(tc.tile_pool(name="recT", bufs=1, space="PSUM"))
                    wh_sb = load_w(rp, wh_d, H, "wh")

                    base = (0, 32)             # psum partition base per dir
                    Cst = [rp.tile([16, H], f32, tag=f"C{d}", name=f"C{d}") for d in range(2)]
                    hT8 = [rp.tile([128, SPD * 32], bf16, tag=f"hT8_{d}", name=f"hT8_{d}")
                           for d in range(2)]
                    for d in range(2):
                        nc.vector.memset(Cst[d][:], 0.0)

                    xgt = [None, None]

                    def step(d, s):
                        tt = s if d == 0 else T - 1 - s
                        j = s % SPD
                        if j == 0:
                            blk = (s // SPD) * SPD * 16
                            xgt[d] = xp.tile([16, SPD * G4], bf16, tag=f"xgt{d}", name=f"xgt{d}_{s}")
                            if d == 0:
                                src = xg_d[0, blk : blk + SPD * 16, :]
                            else:
                                hi = NTOK - blk
                                src = xg_d[1, hi - SPD * 16 : hi, :]
                            nc.sync.dma_start(
                                xgt[d][:].rearrange("b (j c) -> b j c", j=SPD),
                                src.rearrange("(j b) c -> b j c", b=16))
                        # column block of this step inside the batched tile
                        cb = (j if d == 0 else SPD - 1 - j) * G4
                        g = gp.tile([48, G4], f32, tag=f"g{d}", name=f"g{d}_{s}")
                        bp = base[d]
                        tpos = (0, bp)
                        first = s == 0
                        for nb in range(2):
                            nc.tensor.matmul(
                                g[bp : bp + 16, nb * 512 : (nb + 1) * 512],
                                lhsT=id16b[:],
                                rhs=xgt[d][:, cb + nb * 512 : cb + (nb + 1) * 512],
                                start=True, stop=first,
                                tile_position=tpos,
                            )
                        if not first:
                            jp = (s - 1) % SPD
                            for k in range(2):
                                lhs = hT8[d][:, jp * 32 + k * 16 : jp * 32 + (k + 1) * 16]
                                for nb in range(2):
                                    nc.tensor.matmul(
                                        g[bp : bp + 16, nb * 512 : (nb + 1) * 512],
                                        lhsT=lhs,
                                        rhs=wh_sb[d][k][0][:, nb * 512 : (nb + 1) * 512],
                                        start=False, stop=(k == 1),
                                        tile_position=tpos,
                                    )
                        Tall = sp.tile([16, G4], bf16, tag=f"Tall{d}", name=f"Tall{d}_{s}")
                        nc.scalar.activation(Tall[:], g[bp : bp + 16, :], AF.Tanh)
                        A = sp.tile([16, H], f32, tag=f"A{d}", name=f"A{d}_{s}")
                        nc.vector.scalar_tensor_tensor(
                            out=A[:], in0=Tall[:, 256:512], scalar=1.0, in1=Cst[d][:],
                            op0=OP.add, op1=OP.mult)
                        Bv = sp.tile([16, H], bf16, tag=f"Bv{d}", name=f"Bv{d}_{s}")
                        nc.vector.scalar_tensor_tensor(
                            out=Bv[:], in0=Tall[:, 0:256], scalar=1.0, in1=Tall[:, 512:768],
                            op0=OP.add, op1=OP.mult)
                        nc.vector.scalar_tensor_tensor(
                            out=Cst[d][:], in0=A[:], scalar=0.5, in1=Bv[:],
                            op0=OP.mult, op1=OP.add)
                        TC = sp.tile([16, H], bf16, tag=f"TC{d}", name=f"TC{d}_{s}")
                        nc.scalar.activation(TC[:], Cst[d][:], AF.Tanh, scale=0.5)
                        Hh = sp.tile([16, H], bf16, tag=f"Hh{d}", name=f"Hh{d}_{s}")
                        nc.vector.scalar_tensor_tensor(
                            out=Hh[:], in0=Tall[:, 768:1024], scalar=1.0, in1=TC[:],
                            op0=OP.add, op1=OP.mult)
                        tps = tp.tile([128, 32], bf16, tag=f"tps{d}", name=f"tps{d}_{s}")
                        for k in range(2):
                            nc.tensor.transpose(
                                out=tps[:, k * 16 : (k + 1) * 16],
                                in_=Hh[:, k * 128 : (k + 1) * 128],
                                identity=id16b[:])
                        nc.vector.tensor_copy(hT8[d][:, j * 32 : (j + 1) * 32], tps[:])
                        if j == SPD - 1:
                            # flush 8 steps of transposed h to DRAM
                            s0 = s - (SPD - 1)
                            for k in range(2):
                                src = hT8[d][:].rearrange("p (j c) -> p j c", c=32)[:, :, k * 16 : (k + 1) * 16]
                                if d == 0:
                                    nc.sync.dma_start(
                                        houtT_d[0, k, :, s0 * 16 : (s0 + SPD) * 16]
                                        .rearrange("p (j c) -> p j c", c=16),
                                        src)
                                else:
                                    t0 = T - 1 - s  # lowest t in the block
                                    nc.sync.dma_start(
                                        houtT_d[1, k, :, t0 * 16 : (t0 + SPD) * 16]
                                        .rearrange("p (j c) -> p j c", c=16),
                                        src[:, ::-1, :])

                    for s in range(T):
                        step(0, s)
                        step(1, s)

            if 'B' in phases:
                with nc.named_scope("phaseB"):
                    recurrence(xg0_d, w0h_d, hcT0_d)

            # ---------------- Phase C: L1 input proj from hcT0 ---------------------
            def proj_l1(hinT_d, xgo_d):
                with ExitStack() as es:
                    sp = es.enter_context(tc.tile_pool(name="pC", bufs=3))
                    wp = es.enter_context(tc.tile_pool(name="pCw", bufs=1))
                    pp = es.enter_context(tc.tile_pool(name="pCp", bufs=2, space="PSUM"))
                    w1i_sb = load_w(wp, w1i_d, 2 * H, "w1i")
                    b1_sb = wp.tile([128, 2 * G4], f32)
                    nc.sync.dma_start(b1_sb[:], b1_d[:].rearrange("p a b -> p (a b)"))
                    # middle-out order: hcT0[t] for both dirs completes from the
                    # middle of the recurrence outward
                    order = []
                    for i in range(NM):
                        order.append(NM // 2 - 1 - i // 2 if i % 2 == 0 else NM // 2 + i // 2)
                    for m in order:
                        ht = sp.tile([128, 512], bf16, tag="ht")
                        nc.sync.dma_start(
                            ht[:].rearrange("p (d k c) -> p d k c", d=2, k=2),
                            hinT_d[:, :, :, m * 128 : (m + 1) * 128]
                            .transpose([2, 0, 1, 3]))
                        hTm = [ht[:, dk * 128 : (dk + 1) * 128] for dk in range(4)]
                        for d in range(2):
                            ps = pp.tile([128, G4], f32, tag="psxg")
                            for k in range(4):
                                for nb in range(2):
                                    nc.tensor.matmul(
                                        ps[:, nb * 512 : (nb + 1) * 512],
                                        lhsT=hTm[k][:],
                                        rhs=w1i_sb[d][k][0][:, nb * 512 : (nb + 1) * 512],
                                        start=(k == 0), stop=(k == 3),
                                    )
                            ev = sp.tile([128, G4], bf16, tag=f"ev{d}")
                            nc.vector.scalar_tensor_tensor(
                                out=ev[:], in0=ps[:], scalar=0.0,
                                in1=b1_sb[:, d * G4 : (d + 1) * G4],
                                op0=OP.add, op1=OP.add)
                            nc.sync.dma_start(xgo_d[d, m * 128 : (m + 1) * 128, :], ev[:])

            if 'C' in phases:
                with nc.named_scope("phaseC"):
                    proj_l1(hcT0_d, xg1_d)
            if 'D' in phases:
                with nc.named_scope("phaseD"):
                    recurrence(xg1_d, w1h_d, hcT1_d)

            # ---------------- Phase E: FC -> emissions, numE, exp-emissions --------
            ET = cp.tile([C, NTOK], f32)          # exp(e)/C, transposed [C, tok]
            accT = cp.tile([128, NM], f32)
            nlnC = cp.tile([128, 1], f32)
            nc.vector.memset(nlnC[:], -float(np.log(C)))
            if 'E' in phases:
              with nc.named_scope("phaseE"), ExitStack() as es:
                sp = es.enter_context(tc.tile_pool(name="pE", bufs=3))
                pp = es.enter_context(tc.tile_pool(name="pEp", bufs=2, space="PSUM"))
                # ascending order (F consumes t ascending); E only starts after D
                for m in range(NM):
                    ps = pp.tile([128, C], f32, tag="pse")
                    ht = sp.tile([128, 512], bf16, tag="eht")
                    nc.sync.dma_start(
                        ht[:].rearrange("p (d k c) -> p d k c", d=2, k=2),
                        hcT1_d[:, :, :, m * 128 : (m + 1) * 128]
                        .transpose([2, 0, 1, 3]))
                    for dk in range(4):
                        nc.tensor.matmul(
                            ps[:], lhsT=ht[:, dk * 128 : (dk + 1) * 128], rhs=fct_sb[dk][:],
                            start=(dk == 0), stop=(dk == 3))
                    em = sp.tile([128, C], f32, tag="em")
                    nc.vector.scalar_tensor_tensor(
                        out=em[:], in0=ps[:], scalar=0.0, in1=fcb_sb[:],
                        op0=OP.add, op1=OP.add)
                    junk = sp.tile([128, C], f32, tag="junk")
                    nc.vector.scalar_tensor_tensor(
                        out=junk[:], in0=em[:], scalar=0.0, in1=mask_sb[:, m * C : (m + 1) * C],
                        op0=OP.add, op1=OP.mult, accum_out=accT[:, m : m + 1])
                    ee = sp.tile([128, C], f32, tag="ee")
                    nc.scalar.activation(ee[:], em[:], AF.Exp, bias=nlnC[:, :1])
                    tps = pp.tile([128, 128], f32, tag="tp2")
                    nc.tensor.transpose(out=tps[:C, :], in_=ee[:], identity=ident[:])
                    nc.scalar.copy(out=ET[:, m * 128 : (m + 1) * 128], in_=tps[:C, :])

                # numE[b] = sum_p sel[p,b] * rowsum(accT)
                accR = sp.tile([128, 1], f32, tag="accR")
                nc.vector.tensor_reduce(accR[:], accT[:], axis=mybir.AxisListType.X, op=OP.add)
                psn = pp.tile([16, 1], f32, tag="psn")
                nc.tensor.matmul(psn[:], lhsT=sel_sb[:], rhs=accR[:], start=True, stop=True)
                neo = sp.tile([16, 1], f32, tag="neo")
                nc.scalar.copy(out=neo[:], in_=psn[:])
                nc.sync.dma_start(ne_out[:].rearrange("a b -> b a"), neo[:])

            # ---------------- Phase F: CRF forward algorithm (exp domain) ----------
            # Two independent 8-sample chains (col groups 0 and 1).
            if 'F' in phases:
              with nc.named_scope("phaseF"), ExitStack() as es:
                sp = es.enter_context(tc.tile_pool(name="pF", bufs=4))
                pp = es.enter_context(tc.tile_pool(name="pFp", bufs=1, space="PSUM"))
                logacc = cp.tile([1, B_LOC], f32)
                nc.vector.memset(logacc[:], 0.0)
                HB = B_LOC // 2
                a_ch = []
                for ch in range(2):
                    a = sp.tile([C, HB], f32, tag=f"a0_{ch}")
                    nc.vector.tensor_scalar(
                        a[:], ET[:, ch * HB : (ch + 1) * HB], est_sb[:, :1], None, op0=OP.mult)
                    a_ch.append(a)

                def fstep(ch, t, a):
                    bp = 32 * ch
                    ps = pp.tile([52, HB], f32, tag=f"psa{ch}", name=f"psa{ch}_{t}")
                    nc.tensor.matmul(ps[bp : bp + C, :], lhsT=pm_sb[:], rhs=a[:],
                                     start=True, stop=True, tile_position=(0, bp))
                    an = sp.tile([C, HB], f32, tag=f"a{ch}_{t % 3 + 1}")
                    nc.vector.scalar_tensor_tensor(
                        out=an[:], in0=ps[bp : bp + C, :], scalar=0.0,
                        in1=ET[:, t * B_LOC + ch * HB : t * B_LOC + (ch + 1) * HB],
                        op0=OP.add, op1=OP.mult)
                    if t % RESCALE_EVERY == 0:
                        nrm = pp.tile([52, HB], f32, tag=f"nrm{ch}")
                        nc.tensor.matmul(nrm[bp : bp + C, :], lhsT=ones20[:], rhs=an[:],
                                         start=True, stop=True, tile_position=(0, bp))
                        lnn = sp.tile([1, HB], f32, tag=f"lnn{ch}")
                        nc.scalar.activation(lnn[:], nrm[bp : bp + 1, :], AF.Ln)
                        nc.vector.tensor_tensor(
                            out=logacc[:, ch * HB : (ch + 1) * HB],
                            in0=logacc[:, ch * HB : (ch + 1) * HB], in1=lnn[:], op=OP.add)
                        rcp = sp.tile([C, HB], f32, tag=f"rcp{ch}")
                        nc.vector.reciprocal(rcp[:], nrm[bp : bp + C, :])
                        a2 = sp.tile([C, HB], f32, tag=f"ars{ch}")
                        nc.vector.tensor_tensor(out=a2[:], in0=an[:], in1=rcp[:], op=OP.mult)
                        return a2
                    return an

                for t in range(1, T):
                    a_ch[0] = fstep(0, t, a_ch[0])
                    a_ch[1] = fstep(1, t, a_ch[1])

                so = sp.tile([1, B_LOC], f32, tag="so")
                for ch in range(2):
                    af = sp.tile([C, HB], f32, tag=f"af{ch}")
                    nc.vector.tensor_scalar(af[:], a_ch[ch][:], een_sb[:, :1], None, op0=OP.mult)
                    pss = pp.tile([1, HB], f32, tag=f"pss{ch}")
                    nc.tensor.matmul(pss[:], lhsT=ones20[:, :1], rhs=af[:], start=True, stop=True)
                    nc.scalar.copy(out=so[:, ch * HB : (ch + 1) * HB], in_=pss[:])
                nc.sync.dma_start(s_out[:], so[:])
                lao = sp.tile([1, B_LOC], f32, tag="lao")
                nc.vector.tensor_scalar(lao[:], logacc[:], chain_sb[:, :1], None, op0=OP.add)
                nc.sync.dma_start(la_out[:], lao[:])

        for _rep in range(reps):
            whole_model()

    nc.compile()
    return nc


def _prep_host(inputs, T):
    """Host-side weight transforms + per-core in_maps."""
    f32 = np.float32
    from ml_dtypes import bfloat16 as bf16np
    ids_full = np.asarray(inputs["input_ids"]).astype(np.int32)      # [B, T]
    labels = np.asarray(inputs["labels"]).astype(np.int64)           # [B, T]
    emb = np.asarray(inputs["emb"], dtype=f32)
    trans = np.asarray(inputs["transitions"], dtype=f32)
    start = np.asarray(inputs["start_trans"], dtype=f32)
    end = np.asarray(inputs["end_trans"], dtype=f32)

    colscale = np.ones(G4, f32)
    colscale[0:256] = 0.5       # i
    colscale[256:512] = 0.5     # f
    colscale[768:1024] = 0.5    # o

    def prep_layer(wi, wh, bi, bh, in_scale):
        # wi [2, 4H, in], wh [2, 4H, H] -> transposed, scaled
        wiT = np.ascontiguousarray(np.transpose(wi, (0, 2, 1))).astype(f32)
        whT = np.ascontiguousarray(np.transpose(wh, (0, 2, 1))).astype(f32)
        wiT = wiT * in_scale * colscale[None, None, :]
        whT = whT * 0.5 * colscale[None, None, :]
        b = (np.asarray(bi, f32) + np.asarray(bh, f32)) * colscale[None, :]
        # bias as an extra contraction row of wi
        wiTb = np.concatenate([wiT, b[:, None, :]], axis=1)
        return wiTb, whT, b

    w0i, w0h, b0 = prep_layer(inputs["w_ih_l0"], inputs["w_hh_l0"],
                              inputs["b_ih_l0"], inputs["b_hh_l0"], 1.0)
    w1i, w1h, b1 = prep_layer(inputs["w_ih_l1"], inputs["w_hh_l1"],
                              inputs["b_ih_l1"], inputs["b_hh_l1"], 0.5)
    w1i = w1i[:, :-1, :]        # L1 bias is added on-device via b1r instead
    fcT = (np.asarray(inputs["fc_w"], f32).T * 0.5).astype(f32)      # [2H, C]
    fcb = np.asarray(inputs["fc_b"], f32)

    b1r = np.broadcast_to(b1[None, :, :], (128, 2, G4)).copy()
    fcbr = np.broadcast_to(fcb[None, :], (128, C)).copy()
    Pm = np.exp(trans).astype(f32)
    est = np.exp(start).astype(f32).reshape(C, 1)
    een = np.exp(end).astype(f32).reshape(C, 1)
    selm = (np.arange(128)[:, None] % B_LOC == np.arange(B_LOC)[None, :]).astype(f32)

    NTOK = T * B_LOC
    NM = NTOK // 128
    in_maps = []
    host_num = np.zeros(B, np.float64)
    for c in range(N_CORES):
        bs = slice(c * B_LOC, (c + 1) * B_LOC)
        ids_c = ids_full[bs, :T].T.reshape(NTOK)                     # (t,b) t-major
        ids_tile = ids_c.reshape(NM, 128).T.copy()                   # [128, NM]
        lab_c = labels[bs, :T].T.reshape(NTOK)                       # token (t,b)
        maskE = np.zeros((128, NM * C), f32)
        toks = np.arange(NTOK)
        maskE[toks % 128, (toks // 128) * C + lab_c] = 1.0
        in_maps.append({
            "ids": ids_tile.astype(np.int32), "emb": emb,
            "w0i": w0i.astype(bf16np), "w0h": w0h.astype(bf16np),
            "w1i": w1i.astype(bf16np), "w1h": w1h.astype(bf16np),
            "b1r": b1r,
            "fcT": fcT.astype(bf16np), "fcbr": fcbr,
            "maskE": maskE, "selm": selm,
            "Pm": Pm, "estart": est, "eend": een,
            "chain": np.zeros((1, 1), f32),
        })
        # host part of gold-path score (pure input data)
        lb = labels[bs, :T]
        host_num[c * B_LOC:(c + 1) * B_LOC] = (
            start[lb[:, 0]].astype(np.float64)
            + trans[lb[:, :-1], lb[:, 1:]].sum(-1)
            + end[lb[:, -1]]
        )
    return in_maps, host_num


def _run(inputs, T):
    from concourse.bass_utils import run_bass_kernel_spmd

    if T not in _COMPILED:
        _COMPILED[T] = _build(T)
    nc = _COMPILED[T]
    in_maps, host_num = _prep_host(inputs, T)
    res = run_bass_kernel_spmd(nc, in_maps, core_ids=list(range(N_CORES)))
    total = 0.0
    for c in range(N_CORES):
        r = res.results[c]
        S = r["S_out"].reshape(B_LOC).astype(np.float64)
        numE = r["numE_out"].reshape(B_LOC).astype(np.float64)
        logacc = r["logacc_out"].reshape(B_LOC).astype(np.float64)
        logZ = np.log(S) + logacc + T * np.log(C)
        num = host_num[c * B_LOC:(c + 1) * B_LOC] + numE
        total += (logZ - num).sum()
    return np.float32(total)


def kernel(**inputs):
    return _run(inputs, T_FULL)
